# revision 47
# baseline (speedup 1.0000x reference)
"""GroupARouter MoE-routing kernel for 8 Trainium2 NeuronCores.

Strategy: data-parallel over batch B=8 (one batch per core). Host preps
per-core transposed gate input [tokens.T; xyz.T; ones] so the PE never
transposes the 16 MiB token matrix. Device computes spatial dist^2
directly in expert-major layout via 4-concurrent tile_position matmuls
(xyz/ones/|x|^2 rows folded into a K=20 stationary), kicks off the
global-mean AllReduce early so it hides under the fp32 gate-MLP stream,
emits content logits as one [8,512] W2-stationary matmul per chunk with
DMA partition-remap into expert-major, folds b2 into the sigmoid bias
and bisection init, runs the exact per-expert top-k (k=4096) threshold
via verified-(lo,w) bisection with a DVE/ACT split count and fp16 count
reduce, and a token-major floor/cap/combine epilogue with division-free
Newton reciprocals and contiguous raw output DMA (host un-shuffles).
"""
import numpy as np

B, N, D, E, TOP_K = 8, 16384, 256, 8, 2
H = D // 2                      # 128
K_SEL = N * TOP_K // E          # 4096
FLOOR = min(0.05, 0.15 / 4)     # 0.0375
ALPHA = FLOOR * E               # 0.3
CAP_LOW, CAP_HIGH, T_MAX = 0.5, 0.6, 1000
NCORES = 8
CH = 512                        # tokens per MLP chunk
NCHUNK = N // CH                # 32
CQ = N // 128                   # 128 column-chunks of 128 tokens
NJ = CQ // 16                   # 8 transpose blocks
BISECT_ITERS = 26
SAR_RANGE = 16.0                # logits guaranteed within [-16, 16]

_CACHE = {}


def _build():
    import concourse.bacc as bacc
    import concourse.mybir as mybir
    import concourse.tile as tile

    F32 = mybir.dt.float32
    F16 = mybir.dt.float16
    I32 = mybir.dt.int32
    AF = mybir.ActivationFunctionType
    ALU = mybir.AluOpType

    nc = bacc.Bacc("TRN2", target_bir_lowering=False, debug=False,
                   num_devices=NCORES)

    # ---- DRAM I/O
    d_gT = nc.dram_tensor("gT", (D + 4, N), F32, kind="ExternalInput")
    d_XQ = nc.dram_tensor("XQ", (20, NCHUNK * 128), F32, kind="ExternalInput")
    d_W4 = nc.dram_tensor("W4", (20, 32), F32, kind="ExternalInput")
    d_tb = nc.dram_tensor("tb", (1, 1), I32, kind="ExternalInput")
    d_W1b = nc.dram_tensor("W1b", (D + 4, H), F32, kind="ExternalInput")
    d_W2 = nc.dram_tensor("W2", (H, E), F32, kind="ExternalInput")
    d_B2em = nc.dram_tensor("B2em", (128, 1), F32, kind="ExternalInput")
    d_Bbc16 = nc.dram_tensor("Bbc16", (128, 128), F16, kind="ExternalInput")
    d_id = nc.dram_tensor("ident", (128, 128), F32, kind="ExternalInput")
    o_disp = nc.dram_tensor("disp", (128, NJ * 128), F32,
                            kind="ExternalOutput")
    o_comb = nc.dram_tensor("comb", (128, NJ * 128), F32,
                            kind="ExternalOutput")
    cc_in = nc.dram_tensor("cc_in", (1, 128), F32, kind="Internal")
    cc_out = nc.dram_tensor("cc_out", (1, 128), F32, kind="Internal",
                            addr_space="Shared")

    with tile.TileContext(nc) as tc:
        with (
            tc.tile_pool(name="consts", bufs=1) as cpool,
            tc.tile_pool(name="gin", bufs=12) as gpool,
            tc.tile_pool(name="hgp", bufs=9) as hpool,
            tc.tile_pool(name="stg", bufs=2) as stpool,
            tc.tile_pool(name="store", bufs=1) as tpool,
            tc.tile_pool(name="small", bufs=1) as mpool,
            tc.tile_pool(name="ph3", bufs=1) as xpool,
        ):
            # ---- constants into SBUF
            t_w0 = cpool.tile([128, H], F32)
            t_w1 = cpool.tile([128, H], F32)
            t_w2k = cpool.tile([4, H], F32)
            t_W2 = cpool.tile([H, E], F32)
            t_XQ = cpool.tile([20, NCHUNK * 128], F32)
            t_W4 = cpool.tile([20, 32], F32)
            t_B2em = cpool.tile([128, 1], F32)
            t_Bbc16 = cpool.tile([128, 128], F16)
            t_id = cpool.tile([128, 128], F32)
            nc.sync.dma_start(t_XQ[:], d_XQ[:, :])
            nc.sync.dma_start(t_W4[:], d_W4[:, :])
            nc.sync.dma_start(t_w0[:], d_W1b[0:128, :])
            nc.sync.dma_start(t_w1[:], d_W1b[128:256, :])
            nc.sync.dma_start(t_w2k[:], d_W1b[256:260, :])
            nc.sync.dma_start(t_W2[:], d_W2[:, :])
            nc.sync.dma_start(t_B2em[:], d_B2em[:, :])
            nc.sync.dma_start(t_Bbc16[:], d_Bbc16[:, :])
            nc.sync.dma_start(t_id[:], d_id[:, :])

            t_zeroW = cpool.tile([128, NJ * 128], F32)
            nc.vector.memset(t_zeroW[:], 0.0)
            t_onesH = cpool.tile([128, 640], F32)
            nc.vector.memset(t_onesH[:], 1.0)
            t_mhalf = cpool.tile([128, 1], F32)
            nc.vector.memset(t_mhalf[:], -0.5)

            # cap = 0.5 + 1.1e-3 * t_b, broadcast to all partitions
            t_ti = mpool.tile([1, 1], I32)
            nc.sync.dma_start(t_ti[:], d_tb[:, :])
            t_tf = mpool.tile([1, 1], F32)
            nc.vector.tensor_copy(t_tf[:], t_ti[:])
            t_cap1 = mpool.tile([1, 1], F32)
            nc.vector.tensor_scalar(
                t_cap1[:], t_tf[:], (CAP_HIGH + CAP_LOW) / T_MAX, CAP_LOW,
                op0=ALU.mult, op1=ALU.add)
            t_cap = mpool.tile([128, 1], F32)
            nc.gpsimd.partition_broadcast(t_cap[:], t_cap1[:])

            # ---- expert-major accumulators: partition p = 8*(cq%16)+e,
            #      free column = (cq//16)*128 + tl,  token = cq*128+tl
            EMd = tpool.tile([128, NJ * 128], F32)   # dist^2 then dist
            EMc = tpool.tile([128, NJ * 128], F32)   # content logits

            # ---- phase A: dist^2 via 4-concurrent [20,32]x[20,128]
            #      matmuls per group, then dist + early AllReduce
            with tc.tile_pool(name="ps_d", bufs=2, space="PSUM") as ps_d:
                for g in range(NJ):
                    p_D = ps_d.tile([128, 128], F32)
                    for r in range(4):
                        c = g * 4 + r
                        nc.tensor.matmul(
                            p_D[32 * r:32 * r + 32, :], t_W4[:],
                            t_XQ[:, c * 128:(c + 1) * 128],
                            start=True, stop=True, tile_position=(0, 32 * r),
                            skip_group_check=(r > 0))
                    nc.scalar.copy(EMd[:, g * 128:(g + 1) * 128], p_D[:])

            # dist = d2 * rsqrt(d2), one Newton step on rsqrt (DVE/ACT,
            # overlaps the first MLP chunks below)
            nc.vector.tensor_scalar(EMd[:], EMd[:], 1e-12, None,
                                    op0=ALU.max)
            t_y = tpool.tile([128, NJ * 128], F32)
            nc.scalar.activation(t_y[:], EMd[:], AF.Abs_reciprocal_sqrt)
            t_t = tpool.tile([128, NJ * 128], F32)
            nc.vector.tensor_tensor(t_t[:], t_y[:], t_y[:], op=ALU.mult)
            nc.vector.tensor_tensor(t_t[:], t_t[:], EMd[:], op=ALU.mult)
            nc.vector.tensor_scalar(t_t[:], t_t[:], -0.5, 1.5,
                                    op0=ALU.mult, op1=ALU.add)
            nc.vector.tensor_tensor(t_y[:], t_y[:], t_t[:], op=ALU.mult)
            nc.vector.tensor_tensor(EMd[:], EMd[:], t_y[:], op=ALU.mult)
            t_dsum = mpool.tile([128, 1], F32)
            nc.vector.tensor_reduce(t_dsum[:], EMd[:],
                                    axis=mybir.AxisListType.XY, op=ALU.add)

            # ---- phase B: gate MLP, content -> expert-major via DMA remap.
            # Chunks processed in groups of 4; each group's four [128,8]
            # W2-stationary matmuls go to distinct tile_position col-groups
            # so they run concurrently on the PE. Group tails lag one group
            # so the PE never waits on GELU. The dist-sum matmul +
            # AllReduce kickoff slot in after group 0's heads so the PE
            # does not stall on the DVE sqrt chain. Content remap DMAs ride
            # the ACT hwdge queue, input streams the SP queue.
            with (
                tc.tile_pool(name="ps_h", bufs=4, space="PSUM") as ps_h,
                tc.tile_pool(name="ps_l", bufs=2, space="PSUM") as ps_l,
                tc.tile_pool(name="ps_m", bufs=1, space="PSUM") as ps_m,
            ):
                hgs = {}
                # phase-B chunk (b, h): the 512 tokens with column-block
                # cq % 16 == b and j = cq//16 in [4h, 4h+4) -> content
                # lands as one contiguous [8, 512] expert-major rectangle
                gT_v = d_gT[:, :].rearrange("p (j b tl) -> p j b tl",
                                            j=NJ, b=16)

                def chunk_head(b, h, eng=None):
                    # first chunks load via the idle ACT hwdge queue so
                    # their descriptor generation overlaps the const loads
                    # still issuing on the sync queue
                    eng = eng or nc.sync
                    jsl = slice(4 * h, 4 * h + 4)
                    t_g = gpool.tile([128, 2, 4, 128], F32, tag="gchunk")
                    eng.dma_start(t_g[:, 0, :, :], gT_v[0:128, jsl, b, :])
                    eng.dma_start(t_g[:, 1, :, :],
                                  gT_v[128:256, jsl, b, :])
                    t_x1 = gpool.tile([4, 4, 128], F32, tag="xchunk")
                    eng.dma_start(t_x1[:], gT_v[256:260, jsl, b, :])
                    p_h = ps_h.tile([H, CH], F32)
                    nc.tensor.matmul(p_h[:], t_w0[:],
                                     t_g[:, 0, :, :].rearrange(
                                         "p a b -> p (a b)"),
                                     start=True, stop=False)
                    nc.tensor.matmul(p_h[:], t_w1[:],
                                     t_g[:, 1, :, :].rearrange(
                                         "p a b -> p (a b)"),
                                     start=False, stop=False)
                    nc.tensor.matmul(p_h[:], t_w2k[:],
                                     t_x1[:].rearrange("p a b -> p (a b)"),
                                     start=False, stop=True)
                    t_hg = hpool.tile([H, CH], F32, tag="hg")
                    nc.scalar.activation(t_hg[:], p_h[:], AF.Gelu)
                    hgs[(b, h)] = t_hg

                def group_tail(g):
                    h, b0 = g // 4, 4 * (g % 4)
                    p_L = ps_l.tile([128, CH], F32)
                    for r in range(4):
                        nc.tensor.matmul(
                            p_L[32 * r:32 * r + 8, :], t_W2[:],
                            hgs.pop((b0 + r, h))[:], start=True, stop=True,
                            tile_position=(0, 32 * r),
                            skip_group_check=(r > 0))
                    t_st = stpool.tile([128, CH], F32, tag="stage")
                    nc.vector.tensor_copy(t_st[:], p_L[:])
                    for r in range(4):
                        nc.scalar.dma_start(
                            EMc[8 * (b0 + r):8 * (b0 + r) + 8,
                                h * 512:(h + 1) * 512],
                            t_st[32 * r:32 * r + 8, :])

                def head_g(g):
                    h, b0 = g // 4, 4 * (g % 4)
                    return [(b0 + k, h) for k in range(4)]

                for i, bh in enumerate(head_g(0)[:3]):
                    chunk_head(*bh, eng=nc.scalar if i < 2 else None)
                # dist-sum reduce + AllReduce kickoff (data ready by now)
                p_tot = ps_m.tile([128, 1], F32)
                nc.tensor.matmul(p_tot[:], t_onesH[:, 0:128], t_dsum[:],
                                 start=True, stop=True)
                t_S1 = mpool.tile([1, 1], F32)
                nc.scalar.copy(t_S1[:], p_tot[0:1, :])
                t_S = mpool.tile([1, 128], F32)
                nc.vector.tensor_copy(t_S[:], t_S1[:].broadcast_to((1, 128)))
                nc.sync.dma_start(cc_in[:, :], t_S[:])
                nc.gpsimd.collective_compute(
                    "AllReduce", ALU.add, ins=[cc_in[:, :]],
                    outs=[cc_out[:, :]], replica_groups=[list(range(NCORES))])

                # EM logits (without b2) = beta*dist + content, merged in
                # halves: cols 0:512 depend only on groups 0-3, so that
                # half merges (and its sigmoid) hide under the h=1 groups
                EM = tpool.tile([128, NJ * 128], F32)
                t_sig = xpool.tile([128, NJ * 128], F32)
                t_beta = mpool.tile([128, 1], F32)

                def beta_block():
                    t_Sall = mpool.tile([1, 1], F32)
                    nc.sync.dma_start(t_Sall[:], cc_out[:, 0:1])
                    t_m = mpool.tile([1, 1], F32)
                    nc.vector.tensor_scalar(t_m[:], t_Sall[:],
                                            1.0 / (B * N * E), 1e-6,
                                            op0=ALU.mult, op1=ALU.add)
                    t_rm = mpool.tile([1, 1], F32)
                    nc.vector.reciprocal(t_rm[:], t_m[:])
                    t_mr = mpool.tile([1, 1], F32)
                    nc.vector.tensor_tensor(t_mr[:], t_m[:], t_rm[:],
                                            op=ALU.mult)
                    nc.vector.tensor_scalar(t_mr[:], t_mr[:], -1.0, 2.0,
                                            op0=ALU.mult, op1=ALU.add)
                    t_beta1 = mpool.tile([1, 1], F32)
                    nc.vector.tensor_tensor(t_beta1[:], t_rm[:], t_mr[:],
                                            op=ALU.mult)
                    nc.vector.tensor_scalar(t_beta1[:], t_beta1[:], -1.0,
                                            None, op0=ALU.mult)
                    nc.gpsimd.partition_broadcast(t_beta[:], t_beta1[:])

                def merge_half(hh):
                    hs = slice(hh * 512, (hh + 1) * 512)
                    nc.vector.scalar_tensor_tensor(
                        EM[:, hs], EMd[:, hs], t_beta[:], EMc[:, hs],
                        op0=ALU.mult, op1=ALU.add)
                    nc.scalar.activation(t_sig[:, hs], EM[:, hs],
                                         AF.Sigmoid, bias=t_B2em[:])

                chunk_head(*head_g(0)[3])
                for g in range(1, NCHUNK // 4):
                    for bh in head_g(g):
                        chunk_head(*bh)
                    group_tail(g - 1)
                group_tail(NCHUNK // 4 - 1)
                beta_block()
                merge_half(0)
                merge_half(1)

            # ---- phase C: bisection (b2 folded into sigmoid bias +
            #      threshold init)
            with tc.tile_pool(name="ps_c", bufs=2, space="PSUM") as ps_c:

                # verified-(lo,w) bisection in b2-shifted space
                t_lo = mpool.tile([128, 1], F32)
                nc.vector.tensor_scalar(t_lo[:], t_B2em[:], -1.0, -SAR_RANGE,
                                        op0=ALU.mult, op1=ALU.add)
                t_w = mpool.tile([128, 1], F32)
                nc.vector.memset(t_w[:], SAR_RANGE)
                t_mid = mpool.tile([128, 1], F32)
                t_cntd = mpool.tile([128, 1], F32)
                t_ca = mpool.tile([128, 1], F32)
                t_cnt16 = mpool.tile([128, 1], F16)
                t_ge2 = mpool.tile([128, 1], F32)
                t_junk = tpool.tile([128, 640], F32)
                t_junk2 = tpool.tile([128, 384], F32)
                DW_ = 640          # DVE count columns; ACT gets the rest
                AW_ = NJ * 128 - DW_
                # fold the per-partition +AW_/2 of the ACT count into the
                # >=0 compare: 16 * (AW_/2) == 3072, so compare vs
                # K_SEL - 3072 instead
                CMP_ = float(K_SEL - 16 * (AW_ // 2))
                for it in range(BISECT_ITERS):
                    nc.vector.tensor_tensor(t_mid[:], t_lo[:], t_w[:],
                                            op=ALU.add)
                    # ACT half-count: sum sign(mid - EM) = M - P
                    nc.scalar.activation(t_junk2[:], EM[:, DW_:],
                                         AF.Sign, bias=t_mid[:], scale=-1.0,
                                         accum_out=t_ca[:])
                    nc.vector.scalar_tensor_tensor(
                        t_junk[:], EM[:, 0:DW_], t_mid[:], t_onesH[:],
                        op0=ALU.is_ge, op1=ALU.mult, accum_out=t_cntd[:])
                    # fused: cnt = cnt_dve - 0.5*acc
                    nc.vector.scalar_tensor_tensor(
                        t_cnt16[:], t_ca[:], t_mhalf[:], t_cntd[:],
                        op0=ALU.mult, op1=ALU.add)
                    p_ct = ps_c.tile([128, 1], F32)
                    nc.tensor.matmul(p_ct[:], t_Bbc16[:], t_cnt16[:],
                                     start=True, stop=True)
                    nc.vector.tensor_scalar(t_ge2[:], p_ct[:], CMP_, 2.0,
                                            op0=ALU.is_ge, op1=ALU.mult)
                    nc.vector.tensor_scalar(t_w[:], t_w[:], 0.5, None,
                                            op0=ALU.mult)
                    nc.vector.scalar_tensor_tensor(
                        t_lo[:], t_ge2[:], t_w[:], t_lo[:],
                        op0=ALU.mult, op1=ALU.add)

                # d1 = (EM >= lo) * sigmoid(EM + b2)   (expert-major)
                t_d1 = xpool.tile([128, NJ * 128], F32)
                nc.vector.scalar_tensor_tensor(t_d1[:], EM[:], t_lo[:],
                                               t_sig[:], op0=ALU.is_ge,
                                               op1=ALU.mult)

            # ---- phase D: token-major floor/cap/combine epilogue
            TM = xpool.tile([128, NJ, 16, E], F32)
            TM_f = TM[:].rearrange("p a b c -> p (a b c)")
            d1_v = t_d1[:].rearrange("p (a b) -> p a b", a=NJ)
            with tc.tile_pool(name="ps_o", bufs=4, space="PSUM") as ps_o:
                for j in range(NJ):
                    p_O = ps_o.tile([128, 128], F32)
                    nc.tensor.transpose(p_O[:], d1_v[:, j, :], t_id[:])
                    nc.scalar.activation(
                        TM[:, j, :, :].rearrange("p a b -> p (a b)"), p_O[:],
                        AF.Copy, bias=FLOOR, scale=1.0 - ALPHA)

            t_exc = xpool.tile([128, 128, 8], F32)
            t_exc_f = t_exc[:].rearrange("p a b -> p (a b)")
            nc.vector.scalar_tensor_tensor(t_exc_f, TM_f, t_cap[:],
                                           t_zeroW[:], op0=ALU.subtract,
                                           op1=ALU.max)
            t_capd = xpool.tile([128, 128, 8], F32)
            t_capd_f = t_capd[:].rearrange("p a b -> p (a b)")
            nc.vector.tensor_tensor(t_capd_f, TM_f, t_exc_f, op=ALU.subtract)
            # headroom segsum runs on the otherwise-idle GPSIMD engine,
            # in parallel with the DVE excS chain
            t_negh = xpool.tile([128, 128, 8], F32)
            t_negh_f = t_negh[:].rearrange("p a b -> p (a b)")
            nc.vector.scalar_tensor_tensor(t_negh_f, t_capd_f, t_cap[:],
                                           t_zeroW[:], op0=ALU.subtract,
                                           op1=ALU.min)

            def segsum8(src, tag, eng=None):
                eng = eng or nc.vector
                a = xpool.tile([128, 128, 4], F32, tag=tag + "a")
                eng.tensor_tensor(a[:], src[:, :, 0:4], src[:, :, 4:8],
                                  op=ALU.add)
                b = xpool.tile([128, 128, 2], F32, tag=tag + "b")
                eng.tensor_tensor(b[:], a[:, :, 0:2], a[:, :, 2:4],
                                  op=ALU.add)
                s = xpool.tile([128, 128], F32, tag=tag + "s")
                eng.tensor_tensor(
                    s[:], b[:, :, 0:1].rearrange("p a b -> p (a b)"),
                    b[:, :, 1:2].rearrange("p a b -> p (a b)"), op=ALU.add)
                return s

            def recip_pos(src, tag):
                y = xpool.tile([128, 128], F32, tag=tag + "y")
                nc.scalar.activation(y[:], src[:], AF.Abs_reciprocal_sqrt)
                r = xpool.tile([128, 128], F32, tag=tag + "r")
                nc.vector.tensor_tensor(r[:], y[:], y[:], op=ALU.mult)
                e = xpool.tile([128, 128], F32, tag=tag + "e")
                nc.vector.tensor_tensor(e[:], src[:], r[:], op=ALU.mult)
                nc.vector.tensor_scalar(e[:], e[:], -1.0, 2.0,
                                        op0=ALU.mult, op1=ALU.add)
                nc.vector.tensor_tensor(r[:], r[:], e[:], op=ALU.mult)
                return r

            excS = segsum8(t_exc[:], "ex")
            hS = segsum8(t_negh[:], "hs")
            nc.vector.tensor_scalar(hS[:], hS[:], -1.0, 1e-8,
                                    op0=ALU.mult, op1=ALU.max)
            rH = recip_pos(hS, "rh")
            t_f = xpool.tile([128, 128], F32)
            nc.vector.tensor_tensor(t_f[:], excS[:], rH[:], op=ALU.mult)
            f_bc = t_f[:].unsqueeze(2).broadcast_to((128, 128, 8))
            t_tmp = xpool.tile([128, 128, 8], F32)
            nc.vector.tensor_tensor(t_tmp[:], t_negh[:], f_bc, op=ALU.mult)
            t_disp = xpool.tile([128, 128, 8], F32)
            nc.vector.tensor_tensor(t_disp[:], t_capd[:], t_tmp[:],
                                    op=ALU.subtract)
            disp_f = t_disp[:].rearrange("p a b -> p (a b)")
            nc.sync.dma_start(o_disp[:, 0:512], disp_f[:, 0:512])
            nc.scalar.dma_start(o_disp[:, 512:1024], disp_f[:, 512:1024])

            dS = segsum8(t_disp[:], "ds")
            nc.vector.tensor_scalar(dS[:], dS[:], 1e-8, None, op0=ALU.add)
            rD = recip_pos(dS, "rd")
            rD_bc = rD[:].unsqueeze(2).broadcast_to((128, 128, 8))
            t_comb = xpool.tile([128, 128, 8], F32)
            nc.vector.tensor_tensor(t_comb[:], t_disp[:], rD_bc, op=ALU.mult)
            comb_f = t_comb[:].rearrange("p a b -> p (a b)")
            nc.sync.dma_start(o_comb[:, 0:512], comb_f[:, 0:512])
            nc.scalar.dma_start(o_comb[:, 512:1024], comb_f[:, 512:1024])

    nc.compile()
    return nc


def kernel(tokens, spatial_xyz, W1, b1, W2, b2, centers, t):
    tokens = np.ascontiguousarray(np.asarray(tokens, np.float32))
    xyz = np.ascontiguousarray(np.asarray(spatial_xyz, np.float32))
    W1 = np.asarray(W1, np.float32)
    b1 = np.asarray(b1, np.float32)
    W2 = np.asarray(W2, np.float32)
    b2 = np.asarray(b2, np.float32)
    centers = np.asarray(centers, np.float32)
    t = np.asarray(t).astype(np.int32)

    from concourse import bass_utils
    if "nc" not in _CACHE:
        _CACHE["nc"] = _build()
    nc = _CACHE["nc"]

    ident = np.eye(128, dtype=np.float32)
    pe = np.arange(128)
    Bbc16 = (pe[:, None] % 8 == pe[None, :] % 8).astype(np.float16)
    B2em = np.ascontiguousarray(b2[pe % 8][:, None].astype(np.float32))
    W1b = np.zeros((D + 4, H), np.float32)
    W1b[:D + 3] = W1
    W1b[D + 3] = b1
    cc = (centers.astype(np.float64) ** 2).sum(-1).astype(np.float32)
    W4 = np.zeros((20, 32), np.float32)
    for q in range(4):
        W4[q * 5 + 0:q * 5 + 3, q * 8:(q + 1) * 8] = -2.0 * centers.T
        W4[q * 5 + 3, q * 8:(q + 1) * 8] = cc
        W4[q * 5 + 4, q * 8:(q + 1) * 8] = 1.0
    W2c = np.ascontiguousarray(W2)

    in_maps = []
    for bi in range(B):
        gT = np.empty((D + 4, N), np.float32)
        gT[0:D] = tokens[bi].T
        gT[D:D + 3] = xyz[bi].T
        gT[D + 3] = 1.0
        xx = (xyz[bi].astype(np.float64) ** 2).sum(-1).astype(np.float32)
        xyzT = xyz[bi].T.reshape(3, NCHUNK, 4, 128)       # d, c, q, tl
        xxr = xx.reshape(NCHUNK, 4, 128)                  # c, q, tl
        XQ = np.empty((4, 5, NCHUNK, 128), np.float32)    # q, d5, c, tl
        XQ[:, 0:3] = xyzT.transpose(2, 0, 1, 3)
        XQ[:, 3] = 1.0
        XQ[:, 4] = xxr.transpose(1, 0, 2)
        XQ = np.ascontiguousarray(XQ.reshape(20, NCHUNK * 128))
        in_maps.append(dict(
            gT=gT, XQ=XQ, W4=W4, tb=np.array([[t[bi]]], np.int32), W1b=W1b,
            W2=W2c, B2em=B2em, Bbc16=Bbc16, ident=ident))

    import os
    trace = os.environ.get("KERNEL_TRACE", "0") == "1"
    res = bass_utils.run_bass_kernel_spmd(nc, in_maps, list(range(NCORES)),
                                          trace=trace)
    _CACHE["exec_time_ns"] = getattr(res, "exec_time_ns", None)
    _CACHE["last_res"] = res

    def unshuffle(raw):
        # raw[tl, (a, e)] with token = a*128 + tl
        return np.ascontiguousarray(
            raw.reshape(128, 128, E).transpose(1, 0, 2).reshape(N, E))

    disp = np.stack([unshuffle(r["disp"]) for r in res.results])
    comb = np.stack([unshuffle(r["comb"]) for r in res.results])
    return disp, comb


if __name__ == "__main__":
    rng = np.random.default_rng(0)
    ins = dict(
        tokens=rng.standard_normal((B, N, D)).astype(np.float32),
        spatial_xyz=rng.standard_normal((B, N, 3)).astype(np.float32),
        W1=(rng.standard_normal((D + 3, H)) / np.sqrt(D + 3)).astype(np.float32),
        b1=np.zeros(H, np.float32),
        W2=(rng.standard_normal((H, E)) / np.sqrt(H)).astype(np.float32),
        b2=np.zeros(E, np.float32),
        centers=(rng.standard_normal((E, 3)) * 10).astype(np.float32),
        t=rng.integers(0, T_MAX, B).astype(np.int32),
    )
    d, c = kernel(**ins)
    print("disp", d.shape, d.dtype, "comb", c.shape, c.dtype)


# revision 49
# speedup vs baseline: 1.0291x; 1.0291x over previous
"""GroupARouter MoE-routing kernel for 8 Trainium2 NeuronCores.

Strategy: data-parallel over batch B=8 (one batch per core). Host preps
per-core transposed gate input [tokens.T; xyz.T; ones] so the PE never
transposes the 16 MiB token matrix. Device computes spatial dist^2
directly in expert-major layout via 4-concurrent tile_position matmuls
(xyz/ones/|x|^2 rows folded into a K=20 stationary), kicks off the
global-mean AllReduce early so it hides under the fp32 gate-MLP stream,
emits content logits as one [8,512] W2-stationary matmul per chunk with
DMA partition-remap into expert-major, folds b2 into the sigmoid bias
and bisection init, runs the exact per-expert top-k (k=4096) threshold
via verified-(lo,w) bisection with a DVE/ACT split count and fp16 count
reduce, and a token-major floor/cap/combine epilogue with division-free
Newton reciprocals and contiguous raw output DMA (host un-shuffles).
"""
import numpy as np

B, N, D, E, TOP_K = 8, 16384, 256, 8, 2
H = D // 2                      # 128
K_SEL = N * TOP_K // E          # 4096
FLOOR = min(0.05, 0.15 / 4)     # 0.0375
ALPHA = FLOOR * E               # 0.3
CAP_LOW, CAP_HIGH, T_MAX = 0.5, 0.6, 1000
NCORES = 8
CH = 512                        # tokens per MLP chunk
NCHUNK = N // CH                # 32
CQ = N // 128                   # 128 column-chunks of 128 tokens
NJ = CQ // 16                   # 8 transpose blocks
BISECT_ITERS = 26
SAR_RANGE = 16.0                # logits guaranteed within [-16, 16]

_CACHE = {}


def _build():
    import concourse.bacc as bacc
    import concourse.mybir as mybir
    import concourse.tile as tile

    F32 = mybir.dt.float32
    F16 = mybir.dt.float16
    I32 = mybir.dt.int32
    AF = mybir.ActivationFunctionType
    ALU = mybir.AluOpType

    nc = bacc.Bacc("TRN2", target_bir_lowering=False, debug=False,
                   num_devices=NCORES)

    # ---- DRAM I/O
    d_gT = nc.dram_tensor("gT", (D + 4, N), F32, kind="ExternalInput")
    d_XQ = nc.dram_tensor("XQ", (20, NCHUNK * 128), F32, kind="ExternalInput")
    d_W4 = nc.dram_tensor("W4", (20, 32), F32, kind="ExternalInput")
    d_tb = nc.dram_tensor("tb", (1, 1), I32, kind="ExternalInput")
    d_W1b = nc.dram_tensor("W1b", (D + 4, H), F32, kind="ExternalInput")
    d_W2 = nc.dram_tensor("W2", (H, E), F32, kind="ExternalInput")
    d_B2em = nc.dram_tensor("B2em", (128, 1), F32, kind="ExternalInput")
    d_Bbc16 = nc.dram_tensor("Bbc16", (128, 128), F16, kind="ExternalInput")
    d_id = nc.dram_tensor("ident", (128, 128), F32, kind="ExternalInput")
    o_disp = nc.dram_tensor("disp", (128, NJ * 128), F32,
                            kind="ExternalOutput")
    o_comb = nc.dram_tensor("comb", (128, NJ * 128), F32,
                            kind="ExternalOutput")
    cc_in = nc.dram_tensor("cc_in", (1, 128), F32, kind="Internal")
    cc_out = nc.dram_tensor("cc_out", (1, 128), F32, kind="Internal",
                            addr_space="Shared")

    with tile.TileContext(nc) as tc:
        with (
            tc.tile_pool(name="consts", bufs=1) as cpool,
            tc.tile_pool(name="gin", bufs=12) as gpool,
            tc.tile_pool(name="hgp", bufs=9) as hpool,
            tc.tile_pool(name="stg", bufs=2) as stpool,
            tc.tile_pool(name="store", bufs=1) as tpool,
            tc.tile_pool(name="small", bufs=1) as mpool,
            tc.tile_pool(name="ph3", bufs=1) as xpool,
        ):
            # ---- constants into SBUF
            t_w0 = cpool.tile([128, H], F32)
            t_w1 = cpool.tile([128, H], F32)
            t_w2k = cpool.tile([4, H], F32)
            t_W2 = cpool.tile([H, E], F32)
            t_XQ = cpool.tile([20, NCHUNK * 128], F32)
            t_W4 = cpool.tile([20, 32], F32)
            t_B2em = cpool.tile([128, 1], F32)
            t_Bbc16 = cpool.tile([128, 128], F16)
            t_id = cpool.tile([128, 128], F32)
            nc.sync.dma_start(t_XQ[:], d_XQ[:, :])
            nc.sync.dma_start(t_W4[:], d_W4[:, :])
            nc.sync.dma_start(t_w0[:], d_W1b[0:128, :])
            nc.sync.dma_start(t_w1[:], d_W1b[128:256, :])
            nc.sync.dma_start(t_w2k[:], d_W1b[256:260, :])
            nc.sync.dma_start(t_W2[:], d_W2[:, :])
            nc.sync.dma_start(t_B2em[:], d_B2em[:, :])
            nc.sync.dma_start(t_Bbc16[:], d_Bbc16[:, :])
            nc.sync.dma_start(t_id[:], d_id[:, :])

            t_zeroW = cpool.tile([128, NJ * 128], F32)
            nc.vector.memset(t_zeroW[:], 0.0)
            t_onesH = cpool.tile([128, 640], F32)
            nc.vector.memset(t_onesH[:], 1.0)
            t_mhalf = cpool.tile([128, 1], F32)
            nc.vector.memset(t_mhalf[:], -0.5)

            # cap = 0.5 + 1.1e-3 * t_b, broadcast to all partitions
            t_ti = mpool.tile([1, 1], I32)
            nc.sync.dma_start(t_ti[:], d_tb[:, :])
            t_tf = mpool.tile([1, 1], F32)
            nc.vector.tensor_copy(t_tf[:], t_ti[:])
            t_cap1 = mpool.tile([1, 1], F32)
            nc.vector.tensor_scalar(
                t_cap1[:], t_tf[:], (CAP_HIGH + CAP_LOW) / T_MAX, CAP_LOW,
                op0=ALU.mult, op1=ALU.add)
            t_cap = mpool.tile([128, 1], F32)
            nc.gpsimd.partition_broadcast(t_cap[:], t_cap1[:])

            # ---- expert-major accumulators: partition p = 8*(cq%16)+e,
            #      free column = (cq//16)*128 + tl,  token = cq*128+tl
            EMd = tpool.tile([128, NJ * 128], F32)   # dist^2 then dist
            EMc = tpool.tile([128, NJ * 128], F32)   # content logits

            # ---- phase A: dist^2 via 4-concurrent [20,32]x[20,128]
            #      matmuls per group, then dist + early AllReduce
            with tc.tile_pool(name="ps_d", bufs=2, space="PSUM") as ps_d:
                for g in range(NJ):
                    p_D = ps_d.tile([128, 128], F32)
                    for r in range(4):
                        c = g * 4 + r
                        nc.tensor.matmul(
                            p_D[32 * r:32 * r + 32, :], t_W4[:],
                            t_XQ[:, c * 128:(c + 1) * 128],
                            start=True, stop=True, tile_position=(0, 32 * r),
                            skip_group_check=(r > 0))
                    nc.scalar.copy(EMd[:, g * 128:(g + 1) * 128], p_D[:])

            # dist = d2 * rsqrt(d2), one Newton step on rsqrt (DVE/ACT,
            # overlaps the first MLP chunks below)
            nc.vector.tensor_scalar(EMd[:], EMd[:], 1e-12, None,
                                    op0=ALU.max)
            t_y = tpool.tile([128, NJ * 128], F32)
            nc.scalar.activation(t_y[:], EMd[:], AF.Abs_reciprocal_sqrt)
            t_t = tpool.tile([128, NJ * 128], F32)
            nc.vector.tensor_tensor(t_t[:], t_y[:], t_y[:], op=ALU.mult)
            nc.vector.tensor_tensor(t_t[:], t_t[:], EMd[:], op=ALU.mult)
            nc.vector.tensor_scalar(t_t[:], t_t[:], -0.5, 1.5,
                                    op0=ALU.mult, op1=ALU.add)
            nc.vector.tensor_tensor(t_y[:], t_y[:], t_t[:], op=ALU.mult)
            nc.vector.tensor_tensor(EMd[:], EMd[:], t_y[:], op=ALU.mult)
            t_dsum = mpool.tile([128, 1], F32)
            nc.vector.tensor_reduce(t_dsum[:], EMd[:],
                                    axis=mybir.AxisListType.XY, op=ALU.add)

            # ---- phase B: gate MLP, content -> expert-major via DMA remap.
            # Chunks processed in groups of 4; each group's four [128,8]
            # W2-stationary matmuls go to distinct tile_position col-groups
            # so they run concurrently on the PE. Group tails lag one group
            # so the PE never waits on GELU. The dist-sum matmul +
            # AllReduce kickoff slot in after group 0's heads so the PE
            # does not stall on the DVE sqrt chain. Content remap DMAs ride
            # the ACT hwdge queue, input streams the SP queue.
            with (
                tc.tile_pool(name="ps_h", bufs=4, space="PSUM") as ps_h,
                tc.tile_pool(name="ps_l", bufs=2, space="PSUM") as ps_l,
                tc.tile_pool(name="ps_m", bufs=1, space="PSUM") as ps_m,
            ):
                hgs = {}
                # phase-B chunk (b, h): the 512 tokens with column-block
                # cq % 16 == b and j = cq//16 in [4h, 4h+4) -> content
                # lands as one contiguous [8, 512] expert-major rectangle
                gT_v = d_gT[:, :].rearrange("p (j b tl) -> p j b tl",
                                            j=NJ, b=16)

                def chunk_head(b, h):
                    jsl = slice(4 * h, 4 * h + 4)
                    t_g = gpool.tile([128, 2, 4, 128], F32, tag="gchunk")
                    nc.sync.dma_start(t_g[:, 0, :, :], gT_v[0:128, jsl, b, :])
                    nc.sync.dma_start(t_g[:, 1, :, :],
                                      gT_v[128:256, jsl, b, :])
                    t_x1 = gpool.tile([4, 4, 128], F32, tag="xchunk")
                    nc.sync.dma_start(t_x1[:], gT_v[256:260, jsl, b, :])
                    p_h = ps_h.tile([H, CH], F32)
                    nc.tensor.matmul(p_h[:], t_w0[:],
                                     t_g[:, 0, :, :].rearrange(
                                         "p a b -> p (a b)"),
                                     start=True, stop=False)
                    nc.tensor.matmul(p_h[:], t_w1[:],
                                     t_g[:, 1, :, :].rearrange(
                                         "p a b -> p (a b)"),
                                     start=False, stop=False)
                    nc.tensor.matmul(p_h[:], t_w2k[:],
                                     t_x1[:].rearrange("p a b -> p (a b)"),
                                     start=False, stop=True)
                    t_hg = hpool.tile([H, CH], F32, tag="hg")
                    nc.scalar.activation(t_hg[:], p_h[:], AF.Gelu)
                    hgs[(b, h)] = t_hg

                def group_tail(g):
                    h, b0 = g // 4, 4 * (g % 4)
                    p_L = ps_l.tile([128, CH], F32)
                    for r in range(4):
                        nc.tensor.matmul(
                            p_L[32 * r:32 * r + 8, :], t_W2[:],
                            hgs.pop((b0 + r, h))[:], start=True, stop=True,
                            tile_position=(0, 32 * r),
                            skip_group_check=(r > 0))
                    t_st = stpool.tile([128, CH], F32, tag="stage")
                    nc.vector.tensor_copy(t_st[:], p_L[:])
                    for r in range(4):
                        nc.scalar.dma_start(
                            EMc[8 * (b0 + r):8 * (b0 + r) + 8,
                                h * 512:(h + 1) * 512],
                            t_st[32 * r:32 * r + 8, :])

                def head_g(g):
                    h, b0 = g // 4, 4 * (g % 4)
                    return [(b0 + k, h) for k in range(4)]

                for bh in head_g(0)[:3]:
                    chunk_head(*bh)
                # dist-sum reduce + AllReduce kickoff (data ready by now)
                p_tot = ps_m.tile([128, 1], F32)
                nc.tensor.matmul(p_tot[:], t_onesH[:, 0:128], t_dsum[:],
                                 start=True, stop=True)
                t_S1 = mpool.tile([1, 1], F32)
                nc.scalar.copy(t_S1[:], p_tot[0:1, :])
                t_S = mpool.tile([1, 128], F32)
                nc.vector.tensor_copy(t_S[:], t_S1[:].broadcast_to((1, 128)))
                nc.sync.dma_start(cc_in[:, :], t_S[:])
                nc.gpsimd.collective_compute(
                    "AllReduce", ALU.add, ins=[cc_in[:, :]],
                    outs=[cc_out[:, :]], replica_groups=[list(range(NCORES))])

                # EM logits (without b2) = beta*dist + content, merged in
                # halves: cols 0:512 depend only on groups 0-3, so that
                # half merges (and its sigmoid) hide under the h=1 groups
                EM = tpool.tile([128, NJ * 128], F32)
                t_sig = xpool.tile([128, NJ * 128], F32)
                t_beta = mpool.tile([128, 1], F32)

                def beta_block():
                    t_Sall = mpool.tile([1, 1], F32)
                    nc.sync.dma_start(t_Sall[:], cc_out[:, 0:1])
                    t_m = mpool.tile([1, 1], F32)
                    nc.vector.tensor_scalar(t_m[:], t_Sall[:],
                                            1.0 / (B * N * E), 1e-6,
                                            op0=ALU.mult, op1=ALU.add)
                    t_rm = mpool.tile([1, 1], F32)
                    nc.vector.reciprocal(t_rm[:], t_m[:])
                    t_mr = mpool.tile([1, 1], F32)
                    nc.vector.tensor_tensor(t_mr[:], t_m[:], t_rm[:],
                                            op=ALU.mult)
                    nc.vector.tensor_scalar(t_mr[:], t_mr[:], -1.0, 2.0,
                                            op0=ALU.mult, op1=ALU.add)
                    t_beta1 = mpool.tile([1, 1], F32)
                    nc.vector.tensor_tensor(t_beta1[:], t_rm[:], t_mr[:],
                                            op=ALU.mult)
                    nc.vector.tensor_scalar(t_beta1[:], t_beta1[:], -1.0,
                                            None, op0=ALU.mult)
                    nc.gpsimd.partition_broadcast(t_beta[:], t_beta1[:])

                def merge_half(hh):
                    hs = slice(hh * 512, (hh + 1) * 512)
                    nc.vector.scalar_tensor_tensor(
                        EM[:, hs], EMd[:, hs], t_beta[:], EMc[:, hs],
                        op0=ALU.mult, op1=ALU.add)
                    nc.scalar.activation(t_sig[:, hs], EM[:, hs],
                                         AF.Sigmoid, bias=t_B2em[:])

                chunk_head(*head_g(0)[3])
                for g in range(1, NCHUNK // 4):
                    for bh in head_g(g):
                        chunk_head(*bh)
                    group_tail(g - 1)
                group_tail(NCHUNK // 4 - 1)
                beta_block()
                merge_half(0)
                merge_half(1)

            # ---- phase C: bisection (b2 folded into sigmoid bias +
            #      threshold init)
            with tc.tile_pool(name="ps_c", bufs=2, space="PSUM") as ps_c:

                # verified-(lo,w) bisection in b2-shifted space
                t_lo = mpool.tile([128, 1], F32)
                nc.vector.tensor_scalar(t_lo[:], t_B2em[:], -1.0, -SAR_RANGE,
                                        op0=ALU.mult, op1=ALU.add)
                t_w = mpool.tile([128, 1], F32)
                nc.vector.memset(t_w[:], SAR_RANGE)
                t_mid = mpool.tile([128, 1], F32)
                t_cntd = mpool.tile([128, 1], F32)
                t_ca = mpool.tile([128, 1], F32)
                t_cnt16 = mpool.tile([128, 1], F16)
                t_ge2 = mpool.tile([128, 1], F32)
                t_junk = tpool.tile([128, 640], F32)
                t_junk2 = tpool.tile([128, 384], F32)
                DW_ = 640          # DVE count columns; ACT gets the rest
                AW_ = NJ * 128 - DW_
                # fold the per-partition +AW_/2 of the ACT count into the
                # >=0 compare: 16 * (AW_/2) == 3072, so compare vs
                # K_SEL - 3072 instead
                CMP_ = float(K_SEL - 16 * (AW_ // 2))
                for it in range(BISECT_ITERS):
                    nc.vector.tensor_tensor(t_mid[:], t_lo[:], t_w[:],
                                            op=ALU.add)
                    # ACT half-count: sum sign(mid - EM) = M - P
                    nc.scalar.activation(t_junk2[:], EM[:, DW_:],
                                         AF.Sign, bias=t_mid[:], scale=-1.0,
                                         accum_out=t_ca[:])
                    nc.vector.scalar_tensor_tensor(
                        t_junk[:], EM[:, 0:DW_], t_mid[:], t_onesH[:],
                        op0=ALU.is_ge, op1=ALU.mult, accum_out=t_cntd[:])
                    # fused: cnt = cnt_dve - 0.5*acc
                    nc.vector.scalar_tensor_tensor(
                        t_cnt16[:], t_ca[:], t_mhalf[:], t_cntd[:],
                        op0=ALU.mult, op1=ALU.add)
                    p_ct = ps_c.tile([128, 1], F32)
                    nc.tensor.matmul(p_ct[:], t_Bbc16[:], t_cnt16[:],
                                     start=True, stop=True)
                    nc.vector.tensor_scalar(t_ge2[:], p_ct[:], CMP_, 2.0,
                                            op0=ALU.is_ge, op1=ALU.mult)
                    nc.vector.tensor_scalar(t_w[:], t_w[:], 0.5, None,
                                            op0=ALU.mult)
                    nc.vector.scalar_tensor_tensor(
                        t_lo[:], t_ge2[:], t_w[:], t_lo[:],
                        op0=ALU.mult, op1=ALU.add)

                # d1 = (EM >= lo) * sigmoid(EM + b2)   (expert-major)
                t_d1 = xpool.tile([128, NJ * 128], F32)
                nc.vector.scalar_tensor_tensor(t_d1[:], EM[:], t_lo[:],
                                               t_sig[:], op0=ALU.is_ge,
                                               op1=ALU.mult)

            # ---- phase D: token-major floor/cap/combine epilogue
            TM = xpool.tile([128, NJ, 16, E], F32)
            TM_f = TM[:].rearrange("p a b c -> p (a b c)")
            d1_v = t_d1[:].rearrange("p (a b) -> p a b", a=NJ)
            with tc.tile_pool(name="ps_o", bufs=4, space="PSUM") as ps_o:
                for j in range(NJ):
                    p_O = ps_o.tile([128, 128], F32)
                    nc.tensor.transpose(p_O[:], d1_v[:, j, :], t_id[:])
                    nc.scalar.activation(
                        TM[:, j, :, :].rearrange("p a b -> p (a b)"), p_O[:],
                        AF.Copy, bias=FLOOR, scale=1.0 - ALPHA)

            t_exc = xpool.tile([128, 128, 8], F32)
            t_exc_f = t_exc[:].rearrange("p a b -> p (a b)")
            nc.vector.scalar_tensor_tensor(t_exc_f, TM_f, t_cap[:],
                                           t_zeroW[:], op0=ALU.subtract,
                                           op1=ALU.max)
            t_capd = xpool.tile([128, 128, 8], F32)
            t_capd_f = t_capd[:].rearrange("p a b -> p (a b)")
            nc.vector.tensor_tensor(t_capd_f, TM_f, t_exc_f, op=ALU.subtract)
            # headroom segsum runs on the otherwise-idle GPSIMD engine,
            # in parallel with the DVE excS chain
            t_negh = xpool.tile([128, 128, 8], F32)
            t_negh_f = t_negh[:].rearrange("p a b -> p (a b)")
            nc.vector.scalar_tensor_tensor(t_negh_f, t_capd_f, t_cap[:],
                                           t_zeroW[:], op0=ALU.subtract,
                                           op1=ALU.min)

            def segsum8(src, tag, eng=None):
                eng = eng or nc.vector
                a = xpool.tile([128, 128, 4], F32, tag=tag + "a")
                eng.tensor_tensor(a[:], src[:, :, 0:4], src[:, :, 4:8],
                                  op=ALU.add)
                b = xpool.tile([128, 128, 2], F32, tag=tag + "b")
                eng.tensor_tensor(b[:], a[:, :, 0:2], a[:, :, 2:4],
                                  op=ALU.add)
                s = xpool.tile([128, 128], F32, tag=tag + "s")
                eng.tensor_tensor(
                    s[:], b[:, :, 0:1].rearrange("p a b -> p (a b)"),
                    b[:, :, 1:2].rearrange("p a b -> p (a b)"), op=ALU.add)
                return s

            def recip_pos(src, tag):
                y = xpool.tile([128, 128], F32, tag=tag + "y")
                nc.scalar.activation(y[:], src[:], AF.Abs_reciprocal_sqrt)
                r = xpool.tile([128, 128], F32, tag=tag + "r")
                nc.vector.tensor_tensor(r[:], y[:], y[:], op=ALU.mult)
                e = xpool.tile([128, 128], F32, tag=tag + "e")
                nc.vector.tensor_tensor(e[:], src[:], r[:], op=ALU.mult)
                nc.vector.tensor_scalar(e[:], e[:], -1.0, 2.0,
                                        op0=ALU.mult, op1=ALU.add)
                nc.vector.tensor_tensor(r[:], r[:], e[:], op=ALU.mult)
                return r

            excS = segsum8(t_exc[:], "ex")
            hS = segsum8(t_negh[:], "hs")
            nc.vector.tensor_scalar(hS[:], hS[:], -1.0, 1e-8,
                                    op0=ALU.mult, op1=ALU.max)
            rH = recip_pos(hS, "rh")
            t_f = xpool.tile([128, 128], F32)
            nc.vector.tensor_tensor(t_f[:], excS[:], rH[:], op=ALU.mult)
            f_bc = t_f[:].unsqueeze(2).broadcast_to((128, 128, 8))
            t_tmp = xpool.tile([128, 128, 8], F32)
            nc.vector.tensor_tensor(t_tmp[:], t_negh[:], f_bc, op=ALU.mult)
            t_disp = xpool.tile([128, 128, 8], F32)
            nc.vector.tensor_tensor(t_disp[:], t_capd[:], t_tmp[:],
                                    op=ALU.subtract)
            disp_f = t_disp[:].rearrange("p a b -> p (a b)")
            nc.sync.dma_start(o_disp[:, 0:512], disp_f[:, 0:512])
            nc.scalar.dma_start(o_disp[:, 512:1024], disp_f[:, 512:1024])

            dS = segsum8(t_disp[:], "ds")
            nc.vector.tensor_scalar(dS[:], dS[:], 1e-8, None, op0=ALU.add)
            rD = recip_pos(dS, "rd")
            rD_bc = rD[:].unsqueeze(2).broadcast_to((128, 128, 8))
            t_comb = xpool.tile([128, 128, 8], F32)
            nc.vector.tensor_tensor(t_comb[:], t_disp[:], rD_bc, op=ALU.mult)
            comb_f = t_comb[:].rearrange("p a b -> p (a b)")
            nc.sync.dma_start(o_comb[:, 0:512], comb_f[:, 0:512])
            nc.scalar.dma_start(o_comb[:, 512:1024], comb_f[:, 512:1024])

    nc.compile()
    return nc


def kernel(tokens, spatial_xyz, W1, b1, W2, b2, centers, t):
    tokens = np.ascontiguousarray(np.asarray(tokens, np.float32))
    xyz = np.ascontiguousarray(np.asarray(spatial_xyz, np.float32))
    W1 = np.asarray(W1, np.float32)
    b1 = np.asarray(b1, np.float32)
    W2 = np.asarray(W2, np.float32)
    b2 = np.asarray(b2, np.float32)
    centers = np.asarray(centers, np.float32)
    t = np.asarray(t).astype(np.int32)

    from concourse import bass_utils
    if "nc" not in _CACHE:
        _CACHE["nc"] = _build()
    nc = _CACHE["nc"]

    ident = np.eye(128, dtype=np.float32)
    pe = np.arange(128)
    Bbc16 = (pe[:, None] % 8 == pe[None, :] % 8).astype(np.float16)
    B2em = np.ascontiguousarray(b2[pe % 8][:, None].astype(np.float32))
    W1b = np.zeros((D + 4, H), np.float32)
    W1b[:D + 3] = W1
    W1b[D + 3] = b1
    cc = (centers.astype(np.float64) ** 2).sum(-1).astype(np.float32)
    W4 = np.zeros((20, 32), np.float32)
    for q in range(4):
        W4[q * 5 + 0:q * 5 + 3, q * 8:(q + 1) * 8] = -2.0 * centers.T
        W4[q * 5 + 3, q * 8:(q + 1) * 8] = cc
        W4[q * 5 + 4, q * 8:(q + 1) * 8] = 1.0
    W2c = np.ascontiguousarray(W2)

    in_maps = []
    for bi in range(B):
        gT = np.empty((D + 4, N), np.float32)
        gT[0:D] = tokens[bi].T
        gT[D:D + 3] = xyz[bi].T
        gT[D + 3] = 1.0
        xx = (xyz[bi].astype(np.float64) ** 2).sum(-1).astype(np.float32)
        xyzT = xyz[bi].T.reshape(3, NCHUNK, 4, 128)       # d, c, q, tl
        xxr = xx.reshape(NCHUNK, 4, 128)                  # c, q, tl
        XQ = np.empty((4, 5, NCHUNK, 128), np.float32)    # q, d5, c, tl
        XQ[:, 0:3] = xyzT.transpose(2, 0, 1, 3)
        XQ[:, 3] = 1.0
        XQ[:, 4] = xxr.transpose(1, 0, 2)
        XQ = np.ascontiguousarray(XQ.reshape(20, NCHUNK * 128))
        in_maps.append(dict(
            gT=gT, XQ=XQ, W4=W4, tb=np.array([[t[bi]]], np.int32), W1b=W1b,
            W2=W2c, B2em=B2em, Bbc16=Bbc16, ident=ident))

    import os
    trace = os.environ.get("KERNEL_TRACE", "0") == "1"
    res = bass_utils.run_bass_kernel_spmd(nc, in_maps, list(range(NCORES)),
                                          trace=trace)
    _CACHE["exec_time_ns"] = getattr(res, "exec_time_ns", None)
    _CACHE["last_res"] = res

    def unshuffle(raw):
        # raw[tl, (a, e)] with token = a*128 + tl
        return np.ascontiguousarray(
            raw.reshape(128, 128, E).transpose(1, 0, 2).reshape(N, E))

    disp = np.stack([unshuffle(r["disp"]) for r in res.results])
    comb = np.stack([unshuffle(r["comb"]) for r in res.results])
    return disp, comb


if __name__ == "__main__":
    rng = np.random.default_rng(0)
    ins = dict(
        tokens=rng.standard_normal((B, N, D)).astype(np.float32),
        spatial_xyz=rng.standard_normal((B, N, 3)).astype(np.float32),
        W1=(rng.standard_normal((D + 3, H)) / np.sqrt(D + 3)).astype(np.float32),
        b1=np.zeros(H, np.float32),
        W2=(rng.standard_normal((H, E)) / np.sqrt(H)).astype(np.float32),
        b2=np.zeros(E, np.float32),
        centers=(rng.standard_normal((E, 3)) * 10).astype(np.float32),
        t=rng.integers(0, T_MAX, B).astype(np.int32),
    )
    d, c = kernel(**ins)
    print("disp", d.shape, d.dtype, "comb", c.shape, c.dtype)
